# revision 6
# baseline (speedup 1.0000x reference)
"""Trainium2 Bass kernel for nn_Explore_decoder_add (histogram_binning).

Strategy (8 NeuronCores, tensor-parallel on vocab; v2 rewrite):
  - Wec is streamed in bf16 ONLY (no hi/lo split): rel err ~3e-3 vs the
    2e-2 gate, and it halves both HBM traffic and PE LDWEIGHTS time
    versus the fp32-reconstruction scheme.
  - Host pre-transposes x into both layouts the kernel needs (xT [d,b,s]
    for the q matmul, X0/X1 [s,b,d] for c_s) in bf16 - no on-device PE
    transposes of x, same total bytes as one fp32 copy.
  - One DMA per Wec group (7 x 918KB contiguous) instead of 28; DMA
    instruction issue (~0.7us each) was a large serialization cost.
  - Main stream is split: h_t-terms (Wec rows 0:128 x x0) run as W
    arrives, independent of pooling, into persistent PSUM; c_s-terms
    join once the pooled vector is ready.
  - Attention softmax without max-subtraction (|scores| <= sum|wv| ~ 5
    since features are tanh-bounded): scores are computed directly
    transposed ([s,b]) via per-batch 1-column matmuls, the exp-sum via a
    ones-vector matmul, and 1/sum is broadcast across partitions with a
    rank-1 matmul. This removes the SBUF->SBUF redistribute DMA and its
    multi-us DRAIN stall.
  - Histogram penalty: one-hot outer-product matmuls as before, with
    one-hot generation split across the gpsimd and vector engines.
  - exp output is streamed to DRAM in bf16 (host casts + normalizes);
    per-core softmax denominators are reduced per-group and returned.

Host side only shards/pads/re-encodes inputs and unshards the output.
"""

import numpy as np
import ml_dtypes

B, S, D = 16, 200, 128
V = 100000
NCORES = 8
VS = V // NCORES            # 12500 vocab per core
NCHUNK = 98                 # 98 chunks of 128
VSP = NCHUNK * 128          # 12544 padded shard width
SCH0, SCH1 = 128, 72        # token chunks per batch (200 = 128 + 72)
NEG = -1.0e30

GRP = 14                    # chunks per W group
N_GRP = NCHUNK // GRP       # 7 groups
GW = GRP * 128              # 1792 vocab cols per group per half

_prog_cache = {}


def _build_program():
    import concourse.bacc as bacc
    import concourse.mybir as mybir
    import concourse.tile as tile
    from concourse.masks import make_identity

    f32 = mybir.dt.float32
    bf16 = mybir.dt.bfloat16
    i32 = mybir.dt.int32
    OP = mybir.AluOpType
    ACT = mybir.ActivationFunctionType

    nc = bacc.Bacc("TRN2", target_bir_lowering=False, debug=False,
                   num_devices=NCORES)

    # ---- I/O -------------------------------------------------------------
    xt = nc.dram_tensor("xt", (D, B * S), bf16, kind="ExternalInput").ap()
    x0d = nc.dram_tensor("x0d", (SCH0, B * D), bf16,
                         kind="ExternalInput").ap()
    x1d = nc.dram_tensor("x1d", (SCH1, B * D), bf16,
                         kind="ExternalInput").ap()
    ids = nc.dram_tensor("x_ids", (B, S), i32, kind="ExternalInput").ap()
    wq = nc.dram_tensor("Wq", (D, D), bf16, kind="ExternalInput").ap()
    wk = nc.dram_tensor("Wk", (D, D), bf16, kind="ExternalInput").ap()
    wv = nc.dram_tensor("Wv", (D, 1), bf16, kind="ExternalInput").ap()
    bqk = nc.dram_tensor("bqk", (D, 1), f32, kind="ExternalInput").ap()
    becT = nc.dram_tensor("becT", (128, NCHUNK), f32,
                          kind="ExternalInput").ap()
    lo_in = nc.dram_tensor("lo_in", (1, 1), f32, kind="ExternalInput").ap()
    wcat = nc.dram_tensor("wcat", (D, N_GRP * 2 * GW), bf16,
                          kind="ExternalInput").ap()
    out = nc.dram_tensor("out", (128, NCHUNK * B), bf16,
                         kind="ExternalOutput").ap()
    sums_out = nc.dram_tensor("sums_out", (1, B), f32,
                              kind="ExternalOutput").ap()

    with tile.TileContext(nc) as tc:
        with (
            tc.tile_pool(name="sb", bufs=1) as sb,
            tc.tile_pool(name="wpool", bufs=7) as wpool,
            tc.tile_pool(name="ohpool", bufs=32) as ohpool,
            tc.tile_pool(name="scpool", bufs=2) as scpool,
            tc.tile_pool(name="pph", bufs=1, space="PSUM") as pph,
            tc.tile_pool(name="pmcs", bufs=2, space="PSUM") as pmcs,
            tc.tile_pool(name="pp", bufs=2, space="PSUM") as pp,
        ):
            # ---- sync-queue DMAs: xT first, W group 0, X0/X1, W 1..6 ----
            xT = sb.tile([128, B, S], bf16, name="xT")
            d_xt = nc.sync.dma_start(
                out=xT.rearrange("p b s -> p (b s)"), in_=xt[:, :])

            w_tiles = []
            wt0 = wpool.tile([128, 2 * GW], bf16, name="wt", tag="wt")
            d_w0 = nc.sync.dma_start(out=wt0[:, :], in_=wcat[:, 0:2 * GW])
            tile.add_dep_helper(d_w0.ins, d_xt.ins, sync=True,
                                reason="xt before W stream")
            w_tiles.append(wt0)

            X0 = sb.tile([128, B, D], bf16, name="X0")
            d_x0 = nc.sync.dma_start(
                out=X0.rearrange("p b d -> p (b d)"), in_=x0d[:, :])
            tile.add_dep_helper(d_x0.ins, d_w0.ins, sync=True,
                                reason="issue order")
            X1 = sb.tile([SCH1, B, D], bf16, name="X1")
            d_x1 = nc.sync.dma_start(
                out=X1.rearrange("p b d -> p (b d)"), in_=x1d[:, :])
            tile.add_dep_helper(d_x1.ins, d_x0.ins, sync=True,
                                reason="issue order")

            prev = d_x1
            for g in range(1, N_GRP):
                wt = wpool.tile([128, 2 * GW], bf16, name="wt", tag="wt")
                d_w = nc.sync.dma_start(
                    out=wt[:, :], in_=wcat[:, g * 2 * GW:(g + 1) * 2 * GW])
                tile.add_dep_helper(d_w.ins, prev.ins, sync=True,
                                    reason="issue order")
                w_tiles.append(wt)
                prev = d_w

            # ---- gpsimd-queue small loads -------------------------------
            lo_lin = sb.tile([1, 1], f32, name="lo_lin")
            nc.gpsimd.dma_start(out=lo_lin[:, :], in_=lo_in[:, :])
            ids_nat = sb.tile([B, S], i32, name="ids_nat")
            nc.gpsimd.dma_start(out=ids_nat[:, :], in_=ids[:, :])
            wq_sb = sb.tile([D, D], bf16, name="wq_sb")
            nc.gpsimd.dma_start(out=wq_sb[:, :], in_=wq[:, :])
            wk_sb = sb.tile([D, D], bf16, name="wk_sb")
            nc.gpsimd.dma_start(out=wk_sb[:, :], in_=wk[:, :])
            wv_sb = sb.tile([D, 1], bf16, name="wv_sb")
            nc.gpsimd.dma_start(out=wv_sb[:, :], in_=wv[:, :])
            bqk_sb = sb.tile([D, 1], f32, name="bqk_sb")
            nc.gpsimd.dma_start(out=bqk_sb[:, :], in_=bqk[:, :])
            bec_sb = sb.tile([128, NCHUNK], f32, name="bec_sb")
            nc.gpsimd.dma_start(out=bec_sb[:, :], in_=becT[:, :])

            # ---- constants ----------------------------------------------
            ident = sb.tile([128, 128], f32, name="ident")
            make_identity(nc, ident[:, :])
            ones_row = sb.tile([1, 128], f32, name="ones_row")
            nc.gpsimd.memset(ones_row[:, :], 1.0)
            ones_col_bf = sb.tile([128, 1], bf16, name="ones_col_bf")
            nc.gpsimd.memset(ones_col_bf[:, :], 1.0)
            ones_col_f = sb.tile([128, 1], f32, name="ones_col_f")
            nc.gpsimd.memset(ones_col_f[:, :], 1.0)

            iota_p_i = sb.tile([128, 128], i32, name="iota_p_i")
            nc.gpsimd.iota(iota_p_i[:, :], pattern=[[1, 128]],
                           channel_multiplier=0)
            iota_c_i = sb.tile([128, NCHUNK], i32, name="iota_c_i")
            nc.gpsimd.iota(iota_c_i[:, :], pattern=[[1, NCHUNK]],
                           channel_multiplier=0)
            iota_p = sb.tile([128, 128], bf16, name="iota_p")
            nc.gpsimd.tensor_copy(iota_p[:, :], iota_p_i[:, :])
            iota_c = sb.tile([128, NCHUNK], bf16, name="iota_c")
            nc.gpsimd.tensor_copy(iota_c[:, :], iota_c_i[:, :])

            # lo broadcast to [128, 1] via ones matmul
            lops = pp.tile([128, 1], f32, name="lops", tag="pp")
            nc.tensor.matmul(out=lops[:, :], lhsT=ones_row[:, :],
                             rhs=lo_lin[:, :], start=True, stop=True)
            lo_sb = sb.tile([128, 1], f32, name="lo_sb")
            nc.vector.tensor_copy(lo_sb[:, :], lops[:, :])

            # ---- v3 x0 column + k path ----------------------------------
            v3 = sb.tile([128, 3 * B], bf16, name="v3")
            nc.vector.tensor_copy(v3[:, 0:B], xT[:, :, 0])

            kps = pp.tile([128, B], f32, name="kps", tag="pp")
            nc.tensor.matmul(out=kps[:, :], lhsT=wk_sb[:, :],
                             rhs=v3[:, 0:B], start=True, stop=True)
            kTb = sb.tile([128, B], f32, name="kTb")
            nc.vector.tensor_scalar(kTb[:, :], kps[:, :], bqk_sb[:, 0:1],
                                    None, OP.add)

            # ---- ids -> f32 -> transpose to [s, b] ----------------------
            idsf_nat = sb.tile([B, S], f32, name="idsf_nat")
            nc.gpsimd.tensor_copy(idsf_nat[:, :], ids_nat[:, :])
            ids0f = sb.tile([128, B], f32, name="ids0f")
            ids1f = sb.tile([128, B], f32, name="ids1f")
            tid0 = pp.tile([128, B], f32, name="tid0", tag="pp")
            nc.tensor.transpose(out=tid0[:, :], in_=idsf_nat[:, 0:SCH0],
                                identity=ident[0:B, 0:B])
            nc.vector.tensor_copy(ids0f[:, :], tid0[:, :])
            tid1 = pp.tile([SCH1, B], f32, name="tid1", tag="pp")
            nc.tensor.transpose(out=tid1[:, :], in_=idsf_nat[:, SCH0:S],
                                identity=ident[0:B, 0:B])
            nc.vector.tensor_copy(ids1f[0:SCH1, :], tid1[:, :])

            # ---- id -> (p, c) decomposition (gpsimd) --------------------
            prep = []
            for idt in (ids0f, ids1f):
                lv = scpool.tile([128, B], f32, name="lv", tag="lv")
                nc.gpsimd.tensor_scalar(lv[:, :], idt[:, :], lo_sb[:, 0:1],
                                        None, OP.subtract)
                ct = scpool.tile([128, B], f32, name="ct", tag="ct")
                nc.gpsimd.tensor_scalar(ct[:, :], lv[:, :], 1.0 / 128.0,
                                        -0.4999, OP.mult, OP.add)
                ci = scpool.tile([128, B], i32, name="ci", tag="ci")
                nc.gpsimd.tensor_copy(ci[:, :], ct[:, :])
                c_f = scpool.tile([128, B], f32, name="c_f", tag="c_f")
                nc.gpsimd.tensor_copy(c_f[:, :], ci[:, :])
                p_f = scpool.tile([128, B], f32, name="p_f", tag="p_f")
                nc.gpsimd.tensor_scalar(p_f[:, :], c_f[:, :], -128.0, None,
                                        OP.mult)
                nc.gpsimd.tensor_tensor(out=p_f[:, :], in0=p_f[:, :],
                                        in1=lv[:, :], op=OP.add)
                bad = scpool.tile([128, B], f32, name="bad", tag="bad")
                nc.gpsimd.tensor_scalar(bad[:, :], idt[:, :], 1.5, 1000.0,
                                        OP.is_lt, OP.mult)
                p_use = scpool.tile([128, B], f32, name="p_use", tag="pu",
                                    bufs=2)
                nc.gpsimd.tensor_tensor(out=p_use[:, :], in0=p_f[:, :],
                                        in1=bad[:, :], op=OP.add)
                prep.append((p_use, c_f))

            # ---- q matmuls + k add (vector) -----------------------------
            fq = sb.tile([128, B, S], f32, name="fq")
            xTf = xT.rearrange("p b s -> p (b s)")
            fqf = fq.rearrange("p b s -> p (b s)")
            for g in range(8):
                qps = pp.tile([128, 2 * S], f32, name="qps", tag="pp")
                nc.tensor.matmul(out=qps[:, :], lhsT=wq_sb[:, :],
                                 rhs=xTf[:, g * 2 * S:(g + 1) * 2 * S],
                                 start=True, stop=True)
                nc.vector.tensor_tensor(
                    out=fq[:, 2 * g:2 * g + 2, :],
                    in0=qps.rearrange("p (b s) -> p b s", s=S),
                    in1=kTb[:, 2 * g:2 * g + 2].unsqueeze(2).broadcast_to(
                        [128, 2, S]),
                    op=OP.add)

            # ---- h-pass group 0 (as W arrives) --------------------------
            pmh_tiles = []
            for t in range(4):
                pmh_tiles.append(pph.tile([128, 2, GRP, B], f32,
                                          name=f"pmh{t}", tag=f"pmh{t}"))

            def emit_h(g):
                wt = w_tiles[g]
                ph = pmh_tiles[g // 2]
                for j in range(GRP):
                    nc.tensor.matmul(
                        out=ph[:, g % 2, j, :],
                        lhsT=wt[:, j * 128:(j + 1) * 128],
                        rhs=v3[:, 0:B], start=True, stop=True)

            emit_h(0)

            # ---- tanh (2 big ACTs) --------------------------------------
            fT = sb.tile([128, B, S], bf16, name="fT")
            fTf = fT.rearrange("p b s -> p (b s)")
            HALF = B * S // 2
            nc.scalar.activation(out=fTf[:, 0:HALF], in_=fqf[:, 0:HALF],
                                 func=ACT.Tanh)
            nc.scalar.activation(out=fTf[:, HALF:2 * HALF],
                                 in_=fqf[:, HALF:2 * HALF], func=ACT.Tanh)

            # ---- scoresT [s, b] via per-batch 1-col matmuls -------------
            scT0 = pp.tile([128, B], f32, name="scT0", tag="pp")
            scT1 = pp.tile([128, B], f32, name="scT1", tag="pp")
            for b in range(B):
                nc.tensor.matmul(out=scT0[:, b:b + 1],
                                 lhsT=fT[:, b, 0:SCH0], rhs=wv_sb[:, :],
                                 start=True, stop=True)
                nc.tensor.matmul(out=scT1[0:SCH1, b:b + 1],
                                 lhsT=fT[:, b, SCH0:S], rhs=wv_sb[:, :],
                                 start=True, stop=True)

            # exp without max subtraction (|scores| <= sum|wv| ~ 5)
            escT0 = sb.tile([128, B], bf16, name="escT0")
            nc.scalar.activation(out=escT0[:, :], in_=scT0[:, :],
                                 func=ACT.Exp)
            escT1 = sb.tile([128, B], bf16, name="escT1")
            nc.scalar.activation(out=escT1[0:SCH1, :],
                                 in_=scT1[0:SCH1, :], func=ACT.Exp)

            # per-batch sums via ones matmul, then 1/sum
            esum = pp.tile([1, B], f32, name="esum", tag="pp")
            nc.tensor.matmul(out=esum[:, :], lhsT=ones_col_bf[:, :],
                             rhs=escT0[:, :], start=True, stop=False)
            nc.tensor.matmul(out=esum[:, :], lhsT=ones_col_bf[0:SCH1, :],
                             rhs=escT1[0:SCH1, :], start=False, stop=True)

            # ---- c_s (unnormalized) via per-batch matmuls ---------------
            csU = pp.tile([128, B], f32, name="csU", tag="pp")
            for b in range(B):
                nc.tensor.matmul(out=csU[:, b:b + 1], lhsT=X0[:, b, :],
                                 rhs=escT0[:, b:b + 1], start=True,
                                 stop=False)
                nc.tensor.matmul(out=csU[:, b:b + 1],
                                 lhsT=X1[0:SCH1, b, :],
                                 rhs=escT1[0:SCH1, b:b + 1], start=False,
                                 stop=True)

            recip = sb.tile([1, B], f32, name="recip")
            nc.vector.reciprocal(recip[:, :], esum[:, :])
            rbc = pp.tile([128, B], f32, name="rbc", tag="pp")
            nc.tensor.matmul(out=rbc[:, :], lhsT=ones_row[:, :],
                             rhs=recip[:, :], start=True, stop=True)

            csU_sb = sb.tile([128, B], f32, name="csU_sb")
            nc.vector.tensor_copy(csU_sb[:, :], csU[:, :])
            csT = sb.tile([128, B], f32, name="csT")
            nc.vector.tensor_tensor(out=csT[:, :], in0=csU_sb[:, :],
                                    in1=rbc[:, :], op=OP.mult)
            # hi/lo split of csT into v3[:, B:3B]
            res = sb.tile([128, B], f32, name="res")
            nc.vector.tensor_copy(v3[:, B:2 * B], csT[:, :])
            nc.vector.tensor_tensor(out=res[:, :], in0=csT[:, :],
                                    in1=v3[:, B:2 * B], op=OP.subtract)
            nc.vector.tensor_copy(v3[:, 2 * B:3 * B], res[:, :])

            # ---- one-hots (split gpsimd/vector) + histogram matmuls -----
            oh_tiles = []
            for b in range(B):
                eng = nc.gpsimd if b % 2 == 0 else nc.vector
                pair = []
                for ci_, (p_use, c_f) in enumerate(prep):
                    ohp = ohpool.tile([128, 128], bf16, name="ohp",
                                      tag="ohp")
                    eng.tensor_scalar(ohp[:, :], iota_p[:, :],
                                      p_use[:, b:b + 1], NEG,
                                      OP.is_equal, OP.mult)
                    ohc = ohpool.tile([128, NCHUNK], bf16, name="ohc",
                                      tag="ohc")
                    eng.tensor_scalar(ohc[:, :], iota_c[:, :],
                                      c_f[:, b:b + 1], None, OP.is_equal)
                    pair.append((ohp, ohc))
                oh_tiles.append(pair)

            # penalty init (bec broadcast) on gpsimd
            penalty = sb.tile([128, NCHUNK, B], f32, name="penalty")
            nc.gpsimd.tensor_copy(
                penalty[:, :, :],
                bec_sb.unsqueeze(2).broadcast_to([128, NCHUNK, B]))

            # ---- main stream: interleave h-pass, hist, cs-pass ----------
            exp_buf = sb.tile([128, NCHUNK, B], bf16, name="exp_buf")
            partials = sb.tile([128, B, N_GRP], f32, name="partials")

            def emit_hist(b0, b1):
                for b4 in range(b0, b1, 4):
                    hps4 = pp.tile([128, 4, NCHUNK], f32, name="hps4",
                                   tag="pp")
                    for b in range(b4, b4 + 4):
                        for ci_ in range(2):
                            np_ = SCH0 if ci_ == 0 else SCH1
                            ohp, ohc = oh_tiles[b][ci_]
                            nc.tensor.matmul(
                                out=hps4[:, b - b4, :],
                                lhsT=ohp[0:np_, :], rhs=ohc[0:np_, :],
                                start=(ci_ == 0), stop=(ci_ == 1))
                    nc.vector.tensor_tensor(
                        out=penalty[:, :, b4:b4 + 4],
                        in0=penalty[:, :, b4:b4 + 4],
                        in1=hps4.transpose([0, 2, 1]), op=OP.add)

            def emit_cs(g):
                wt = w_tiles[g]
                ps = pmcs.tile([128, GRP, 2, B], f32, name="pmcs",
                               tag="pmcs")
                for j in range(GRP):
                    nc.tensor.matmul(
                        out=ps[:, j, :, :],
                        lhsT=wt[:, GW + j * 128:GW + (j + 1) * 128],
                        rhs=v3[:, B:3 * B], start=True, stop=True)
                # epilogue
                gsl = slice(g * GRP, (g + 1) * GRP)
                ph = pmh_tiles[g // 2]
                scr = scpool.tile([128, GRP, B], f32, name="scr",
                                  tag="scr")
                nc.vector.tensor_tensor(out=scr[:, :, :],
                                        in0=penalty[:, gsl, :],
                                        in1=ps[:, :, 0, :], op=OP.add)
                nc.vector.tensor_tensor(out=scr[:, :, :], in0=scr[:, :, :],
                                        in1=ps[:, :, 1, :], op=OP.add)
                nc.vector.tensor_tensor(out=scr[:, :, :], in0=scr[:, :, :],
                                        in1=ph[:, g % 2, :, :], op=OP.add)
                nc.scalar.activation(out=exp_buf[:, gsl, :],
                                     in_=scr[:, :, :], func=ACT.Exp)
                nc.gpsimd.dma_start(
                    out=out[:, g * GRP * B:(g + 1) * GRP * B],
                    in_=exp_buf[:, gsl, :].rearrange("p c b -> p (c b)"))
                nc.vector.tensor_reduce(
                    out=partials[:, :, g],
                    in_=exp_buf[:, gsl, :].transpose([0, 2, 1]),
                    axis=mybir.AxisListType.X, op=OP.add)

            emit_h(1)
            emit_h(2)
            emit_hist(0, 8)
            emit_hist(8, 16)
            emit_cs(0)
            emit_h(3)
            emit_cs(1)
            emit_h(4)
            emit_cs(2)
            emit_h(5)
            emit_cs(3)
            emit_h(6)
            emit_cs(4)
            emit_cs(5)
            emit_cs(6)

            # ---- per-core softmax denominators --------------------------
            pr = sb.tile([128, B], f32, name="pr")
            nc.vector.tensor_reduce(out=pr[:, :], in_=partials[:, :, :],
                                    axis=mybir.AxisListType.X, op=OP.add)
            tot_ps = pp.tile([1, B], f32, name="tot_ps", tag="pp")
            nc.tensor.matmul(out=tot_ps[:, :], lhsT=ones_col_f[:, :],
                             rhs=pr[:, :], start=True, stop=True)
            sums_sb = sb.tile([1, B], f32, name="sums_sb")
            nc.vector.tensor_copy(sums_sb[:, :], tot_ps[:, :])
            nc.sync.dma_start(out=sums_out[:, :], in_=sums_sb[:, :])

    nc.compile()
    return nc


def _get_program():
    if "nc" not in _prog_cache:
        _prog_cache["nc"] = _build_program()
    return _prog_cache["nc"]


def kernel(x, x_ids, Wq, bq, Wk, bk, Wv, bv, Wec, bec):
    bf16 = ml_dtypes.bfloat16
    x = np.asarray(x, dtype=np.float32)
    ids_np = np.ascontiguousarray(np.asarray(x_ids).astype(np.int32))
    Wqb = np.ascontiguousarray(np.asarray(Wq, dtype=np.float32).astype(bf16))
    Wkb = np.ascontiguousarray(np.asarray(Wk, dtype=np.float32).astype(bf16))
    Wvb = np.ascontiguousarray(np.asarray(Wv, dtype=np.float32).astype(bf16))
    bqk = np.ascontiguousarray(
        (np.asarray(bq, np.float32) + np.asarray(bk, np.float32))[:, None])
    Wec = np.asarray(Wec, dtype=np.float32)
    bec = np.asarray(bec, dtype=np.float32)

    xb = x.astype(bf16)
    xt = np.ascontiguousarray(xb.transpose(2, 0, 1).reshape(D, B * S))
    x0d = np.ascontiguousarray(
        xb[:, 0:SCH0, :].transpose(1, 0, 2).reshape(SCH0, B * D))
    x1d = np.ascontiguousarray(
        xb[:, SCH0:S, :].transpose(1, 0, 2).reshape(SCH1, B * D))

    nc = _get_program()

    in_maps = []
    for r in range(NCORES):
        lo, hi = r * VS, (r + 1) * VS
        sh = np.zeros((2 * D, VSP), np.float32)
        sh[:, :VS] = Wec[:, lo:hi]
        shb = sh.astype(bf16)
        w0 = shb[0:D].reshape(D, N_GRP, GW)
        w1 = shb[D:2 * D].reshape(D, N_GRP, GW)
        wcat = np.ascontiguousarray(
            np.concatenate([w0, w1], axis=2).reshape(D, N_GRP * 2 * GW))
        becp = np.full((VSP,), NEG, np.float32)
        becp[:VS] = bec[lo:hi]
        becT = np.ascontiguousarray(becp.reshape(NCHUNK, 128).T)
        in_maps.append({
            "xt": xt, "x0d": x0d, "x1d": x1d,
            "x_ids": ids_np,
            "Wq": Wqb, "Wk": Wkb, "Wv": Wvb, "bqk": bqk,
            "becT": becT,
            "lo_in": np.full((1, 1), float(lo), np.float32),
            "wcat": wcat,
        })

    from concourse.bass_utils import run_bass_kernel_spmd
    res = run_bass_kernel_spmd(nc, in_maps, core_ids=list(range(NCORES)))

    gsum = np.zeros((B,), np.float32)
    for r in range(NCORES):
        gsum += res.results[r]["sums_out"][0]
    inv = (1.0 / gsum)[:, None].astype(np.float32)
    outp = np.empty((B, V), np.float32)
    for r in range(NCORES):
        o = np.asarray(res.results[r]["out"]).astype(np.float32)
        o = o.reshape(128, NCHUNK, B)
        # out[p, c, b] -> probs[b, c*128 + p]
        shard = o.transpose(2, 1, 0).reshape(B, VSP)[:, :VS]
        outp[:, r * VS:(r + 1) * VS] = shard * inv
    return outp
